# revision 34
# baseline (speedup 1.0000x reference)
"""GNN message-passing kernel for Trainium2 (8 NeuronCores, Bass/Tile).

Implements:
  embeds = node_emb[nodes] + features @ feat_W.T + feat_b
  2 x { agg[dst] += embeds[src];  embeds = relu((embeds+agg) @ conv1_W.T + conv1_b) }
  out = cosine(embeds, pattern_emb[pattern_id])

Distribution: nodes are permuted (in-degree balanced) and sharded over 8 cores
(12544 nodes each).  Edges are partitioned by dst owner; messages are fetched
with GPSIMD dma_gather (4 SWDGE queues) from a replicated bf16 hi|lo DRAM
table rebuilt each round with an ncfw AllGather.

Data layout: everything on-chip is FEATURE-MAJOR with the hi|lo split on
partitions (0:64 = hi contribution, 64:128 = lo contribution); columns are
nodes in (block, slot) order, col = b*128 + p.  The segment-sum matmul is
  psum[hl, dst] += tile[edge, hl]^T @ onehot[edge, dst]
and the hi+lo fold happens for free inside the conv matmul by duplicating
conv1_W rows (wct4).  The only transposes are 98 PE transposes per table
emit (feature-major -> node-major rows).
"""

import os
import sys

sys.path.insert(0, "/opt/trn_rl_repo")

import numpy as np

import concourse.bass as bass
import concourse.bacc as bacc
import concourse.mybir as mybir
import concourse.tile as tile
from concourse import library_config
from concourse.bass_utils import run_bass_kernel_spmd

N = 100000
NP = 100352          # padded: 8 * 12544
W = 8                # cores
SH = NP // W         # 12544 nodes per core
NBLK = SH // 128     # 98 blocks per core
NBG = NP // 128      # 784 global blocks
NCHUNK = 4
CH = NP // NCHUNK    # 25088 rows per gather chunk (int16-safe)
D = 64
DF = 10
VOCAB = 100
EPS = 1e-8
CALL = int(os.environ.get("BASS_GNN_CALL", "768"))   # gather positions per dma_gather call
G_OH = 16            # one-hot tiles built per DVE op
PHASE = os.environ.get("BASS_GNN_PHASE", "full")
TRACE = os.environ.get("BASS_GNN_TRACE", "0") == "1"
GBUFS = int(os.environ.get("BASS_GNN_GBUFS", "24"))
SINGLE_PKT = os.environ.get("BASS_GNN_SINGLE_PKT", "1") == "1"

f32 = mybir.dt.float32
bf16 = mybir.dt.bfloat16
i16 = mybir.dt.int16
Alu = mybir.AluOpType
Act = mybir.ActivationFunctionType


# ----------------------------------------------------------------------------
# Host-side preprocessing: permutation, block assignment, edge streams.
# ----------------------------------------------------------------------------

def _prep(nodes, edges):
    src = edges[:, 0].astype(np.int64)
    dst = edges[:, 1].astype(np.int64)

    indeg = np.zeros(NP, np.int64)
    np.add.at(indeg, dst, 1)
    order = np.argsort(-indeg, kind="stable")
    # round-robin by in-degree over 784 global blocks -> balanced block loads
    gblock = np.empty(NP, np.int64)
    gslot = np.empty(NP, np.int64)
    gblock[order] = np.arange(NP) % NBG
    gslot[order] = np.arange(NP) // NBG

    # pass 1 fixes each node's core (g % W), hence its gather chunk as a src
    # (core pair = c//2).  Pass 2 re-bins nodes WITHIN each core into blocks,
    # greedily capping every (block, chunk) in-edge cell at <=2 tiles (256),
    # which both shrinks S_sub and aligns tile needs across cores.
    core_of_gblock = np.arange(NBG) % W
    c_n = core_of_gblock[gblock]
    prof = np.zeros((NP, NCHUNK), np.int64)
    np.add.at(prof, (dst, c_n[src] // 2), 1)

    b_n = np.empty(NP, np.int64)
    p_n = np.empty(NP, np.int64)
    for c in range(W):
        mine = np.where(c_n == c)[0]
        mine = mine[np.argsort(-prof[mine].sum(1), kind="stable")]
        loads = np.zeros((NBLK, NCHUNK), np.int64)
        fill = np.zeros(NBLK, np.int64)
        for n in mine:
            cand = loads + prof[n]
            # penalize only NEW tile-count increases past 2 -> overflow
            # concentrates in a few already-heavy cells instead of spreading
            cur = np.maximum(-(-loads // 128), 2)
            tinc = np.maximum(-(-cand // 128) - cur, 0).sum(1)
            score = tinc * 1e6 + cand.max(1) * 1.0 + fill * 1e-3
            score[fill >= 128] = np.inf
            b = int(np.argmin(score))
            b_n[n] = b
            p_n[n] = fill[b]
            fill[b] += 1
            loads[b] += prof[n]
        # align the residual >2-tile cells across cores: relabel blocks by
        # their tile-need pattern (same key as the old alignment sort)
        t0 = np.ceil(loads / 128).astype(np.int64)
        key = (t0 * (5 ** np.arange(NCHUNK))[::-1]).sum(1) * 1000 + loads.sum(1) // 16
        relabel = np.empty(NBLK, np.int64)
        relabel[np.argsort(key, kind="stable")] = np.arange(NBLK)
        b_n[mine] = relabel[b_n[mine]]

    # final permutation: pi(n) = core*SH + slot_within_block*NBLK + local_block
    # (p-major layout so SBUF slab [128, 98, 128] maps contiguously to DRAM rows)
    pi = c_n * SH + p_n * NBLK + b_n

    # exact per-(core, local block, chunk) edge counts under final assignment
    e_dc = c_n[dst]
    e_db = b_n[dst]
    e_dp = p_n[dst]
    e_chunk = pi[src] // CH
    Ec = np.zeros((W, NBLK, NCHUNK), np.int64)
    np.add.at(Ec, (e_dc, e_db, e_chunk), 1)
    S_sub = np.ceil(Ec / 128).astype(np.int64).max(0)  # [NBLK, NCHUNK]
    S_sub = np.maximum(S_sub, 1)   # keep schedule uniform & non-degenerate
    assert S_sub.max() <= CALL // 128

    # edge order: (core, block, chunk, src)
    ek = np.lexsort((pi[src], e_chunk, e_db, e_dc))
    src_pi = pi[src][ek]
    dloc_e = e_dp[ek]
    key_cbk = (e_dc[ek] * NBLK + e_db[ek]) * NCHUNK + e_chunk[ek]
    counts = np.zeros(W * NBLK * NCHUNK + 1, np.int64)
    np.add.at(counts, key_cbk + 1, 1)
    starts = np.cumsum(counts)

    # gather-stream offsets: chunk-major, then block, then tile
    TPOS = int(S_sub.sum() * 128)
    NT = TPOS // 128
    sched_off = np.zeros((NCHUNK, NBLK), np.int64)
    off = 0
    for k in range(NCHUNK):
        for b in range(NBLK):
            sched_off[k, b] = off
            off += int(S_sub[b, k]) * 128
    assert off == TPOS

    gidx = np.zeros((W, TPOS), np.int32)
    dloc = np.full((W, TPOS), -1.0, np.float32)   # stream order
    for c in range(W):
        for k in range(NCHUNK):
            for b in range(NBLK):
                i0 = starts[(c * NBLK + b) * NCHUNK + k]
                i1 = starts[(c * NBLK + b) * NCHUNK + k + 1]
                n = i1 - i0
                o = sched_off[k, b]
                assert n <= S_sub[b, k] * 128
                gidx[c, o:o + n] = src_pi[i0:i1] - k * CH
                dloc[c, o:o + n] = dloc_e[i0:i1]
    assert gidx.min() >= 0 and gidx.max() < CH

    # wrapped-16 int16 idx layout (stream order)
    gidx16 = np.zeros((W, 128, TPOS // 16), np.int16)
    for c in range(W):
        g = gidx[c].reshape(-1, 16).T.astype(np.int16)
        gidx16[c] = np.tile(g, (8, 1))

    # consumption order: block-major (b, k, s) -> stream position of each tile
    cons_pos = []
    for b in range(NBLK):
        for k in range(NCHUNK):
            for s in range(int(S_sub[b, k])):
                cons_pos.append(int(sched_off[k, b]) + s * 128)
    cons_pos = np.array(cons_pos, np.int64)
    assert len(cons_pos) == NT
    # dloc in consumption-tile-column layout [128, NT]
    dloc_s = dloc.reshape(W, NT, 128)        # stream tile t at rows [t]
    stream_tile = cons_pos // 128
    dloc_c = dloc_s[:, stream_tile, :].transpose(0, 2, 1).astype(np.float32).copy()

    # per-chunk call plan, issued round-robin across chunks
    ncalls_k = []
    base_k = []
    for k in range(NCHUNK):
        npos = int(S_sub[:, k].sum() * 128)
        base_k.append(int(sched_off[k, 0]))
        ncalls_k.append((npos + CALL - 1) // CALL)
    calls = []   # (k, stream_off, npos) in issue order
    for ci in range(max(ncalls_k)):
        for k in range(NCHUNK):
            if ci < ncalls_k[k]:
                npos_k = int(S_sub[:, k].sum() * 128)
                o = base_k[k] + ci * CALL
                n = min(CALL, base_k[k] + npos_k - o)
                calls.append((k, o, n))

    return {
        "pi": pi, "S_sub": S_sub, "TPOS": TPOS, "NT": NT,
        "gidx16": gidx16, "dloc_c": dloc_c, "calls": calls,
        "sched_off": sched_off, "cons_pos": cons_pos,
    }


# ----------------------------------------------------------------------------
# Kernel builder
# ----------------------------------------------------------------------------

def _build(meta):
    S_sub = meta["S_sub"]
    TPOS = meta["TPOS"]
    NT = meta["NT"]
    calls = meta["calls"]
    cons_pos = meta["cons_pos"]

    nc = bacc.Bacc("TRN2", debug=False, num_swdge_queues=4,
                   dynamic_dma_scratch_size=int(os.environ.get(
                       "BASS_GNN_DMA_SCRATCH", "32768")))

    featT_in = nc.dram_tensor("featT", [DF + 1, SH], bf16, kind="ExternalInput")
    nedup_in = nc.dram_tensor("ne_dup", [128, 128], bf16, kind="ExternalInput")
    wftd_in = nc.dram_tensor("wft_dup", [DF + 1, 128], bf16, kind="ExternalInput")
    wct4_in = nc.dram_tensor("wct4", [128, 128], f32, kind="ExternalInput")
    cb2_in = nc.dram_tensor("cb2", [128, 1], f32, kind="ExternalInput")
    pq_in = nc.dram_tensor("pq", [128, 2], f32, kind="ExternalInput")
    ones128_in = nc.dram_tensor("ones128", [128, 1], f32, kind="ExternalInput")
    onesrow_in = nc.dram_tensor("ones_row", [1, 128], f32, kind="ExternalInput")
    onesrowb_in = nc.dram_tensor("ones_row_bf", [1, 128], bf16, kind="ExternalInput")
    typesb_in = nc.dram_tensor("types_bf", [1, SH], bf16, kind="ExternalInput")
    iotaP_in = nc.dram_tensor("iotaP", [128, 1], f32, kind="ExternalInput")
    identb_in = nc.dram_tensor("identb", [128, 128], bf16, kind="ExternalInput")
    iota_in = nc.dram_tensor("iota_rep", [128, G_OH * 128], bf16, kind="ExternalInput")
    gidx_in = nc.dram_tensor("gidx16", [128, TPOS // 16], i16, kind="ExternalInput")
    dloc_in = nc.dram_tensor("dloc_c", [128, NT], f32, kind="ExternalInput")

    score_out = nc.dram_tensor("score", [128, NBLK], f32, kind="ExternalOutput")
    if PHASE in ("emb0", "agg", "emb1"):
        dbg_out = nc.dram_tensor("dbg", [128, SH], f32, kind="ExternalOutput")

    shard_hl = nc.dram_tensor("shard_hl", [SH, 2 * D], bf16)
    table_hl = nc.dram_tensor("table_hl", [NP, 2 * D], bf16, addr_space="Shared")

    import contextlib

    def allgather_trigger(tcx, cc):
        """Run the AllGather inside a critical section (deferring the wait
        to the consumer stream deadlocks the tile scheduler)."""
        with tcx.tile_critical():
            nc.gpsimd.collective_compute(
                "AllGather", Alu.bypass,
                replica_groups=[list(range(W))],
                ins=[shard_hl.ap().opt()],
                outs=[table_hl.ap().opt()],
            ).then_inc(cc, 1)
            nc.gpsimd.wait_ge(cc, 1)

    # consumption-tile schedule per block: (first_tile_index, total_tiles)
    S_tot = S_sub.sum(1).astype(np.int64)
    blk_t0 = np.concatenate([[0], np.cumsum(S_tot)])[:-1]

    with tile.TileContext(nc) as tc:
        nc.gpsimd.load_library(library_config.mlp)
        with contextlib.ExitStack() as big:
            cpool = big.enter_context(tc.tile_pool(name="consts", bufs=1))
            cc_ag0 = big.enter_context(nc.semaphore())
            cc_ag1 = big.enter_context(nc.semaphore())
            cc_ag = [cc_ag0, cc_ag1]
            slabs = big.enter_context(tc.tile_pool(name="slabs", bufs=1))

            wct4_sb = cpool.tile([128, 128], f32, tag="wct4")
            nc.sync.dma_start(wct4_sb[:], wct4_in[:])
            cb2_sb = cpool.tile([128, 1], f32, tag="cb2")
            nc.sync.dma_start(cb2_sb[:], cb2_in[:])
            pq_sb = cpool.tile([128, 2], f32, tag="pq")
            nc.sync.dma_start(pq_sb[:], pq_in[:])
            ones128_sb = cpool.tile([128, 1], f32, tag="ones128")
            nc.sync.dma_start(ones128_sb[:], ones128_in[:])
            onesrow_sb = cpool.tile([1, 128], f32, tag="onesrow")
            nc.sync.dma_start(onesrow_sb[:], onesrow_in[:])
            onesrowb_sb = cpool.tile([1, 128], bf16, tag="onesrowb")
            nc.sync.dma_start(onesrowb_sb[:], onesrowb_in[:])
            iotaP_sb = cpool.tile([128, 1], f32, tag="iotaP")
            nc.sync.dma_start(iotaP_sb[:], iotaP_in[:])
            identb_sb = cpool.tile([128, 128], bf16, tag="identb")
            nc.sync.dma_start(identb_sb[:], identb_in[:])
            iota_sb = cpool.tile([128, G_OH * 128], bf16, tag="iota")
            nc.sync.dma_start(iota_sb[:], iota_in[:])
            dloc_sb = cpool.tile([128, NT], f32, tag="dloc")
            nc.sync.dma_start(dloc_sb[:], dloc_in[:])
            gidx_sb = cpool.tile([128, TPOS // 16], i16, tag="gidx")
            nc.sync.dma_start(gidx_sb[:], gidx_in[:])

            z0 = slabs.tile([128, SH], bf16, tag="z0")
            z1 = slabs.tile([128, SH], bf16, tag="z1")
            num_slab = slabs.tile([128, NBLK], f32, tag="num")
            nsq_slab = slabs.tile([128, NBLK], f32, tag="nsq")

            def split_hilo(z_slab, src_ps, scp_, cols, wdt, relu):
                """Exact hi|lo split of src_ps into z_slab[:, cols] (bf16).

                hi half (partitions 0:64) and lo half (64:128) of src_ps hold
                the same value v (replicated); store hi = bf16(v) and
                lo = bf16(v - f32(hi)).  Copies/casts run on the Scalar
                engine; subtract on DVE.
                """
                if relu:
                    sf = scp_.tile([128, 512], f32, tag="sf")
                    nc.scalar.activation(sf[:, :wdt], src_ps[:, :wdt],
                                         Act.Relu, bias=cb2_sb[:, :1])
                    v = sf
                else:
                    v = src_ps
                nc.scalar.activation(z_slab[0:64, cols], v[0:64, :wdt],
                                     Act.Copy)
                sb_ = scp_.tile([128, 512], bf16, tag="sb")
                nc.vector.tensor_copy(out=sb_[64:128, :wdt],
                                      in_=v[64:128, :wdt])
                sf2 = scp_.tile([128, 512], f32, tag="sf2")
                nc.scalar.activation(sf2[64:128, :wdt], sb_[64:128, :wdt],
                                     Act.Copy)
                nc.vector.tensor_tensor(
                    out=sf2[64:128, :wdt], in0=v[64:128, :wdt],
                    in1=sf2[64:128, :wdt], op=Alu.subtract)
                nc.scalar.activation(z_slab[64:128, cols], sf2[64:128, :wdt],
                                     Act.Copy)

            def emit_blocks(z_slab, hl, eps, b0, nblks, on_dve=False):
                """PE-transpose blocks b0..b0+nblks of z into the hl slab."""
                for b in range(b0, b0 + nblks):
                    tp = eps.tile([128, 128], bf16, tag="tp")
                    nc.tensor.transpose(
                        tp[:], z_slab[:, b * 128:(b + 1) * 128], identb_sb[:])
                    if on_dve:
                        nc.vector.tensor_copy(out=hl[:, b, :], in_=tp[:])
                    else:
                        nc.scalar.activation(hl[:, b, :], tp[:], Act.Copy)

            # ---------------- Phase 0: initial embeddings (feature-major) ---
            with tc.tile_pool(name="ph0", bufs=3) as ph0, \
                 tc.tile_pool(name="ph0c", bufs=1) as ph0c, \
                 tc.tile_pool(name="ph0s", bufs=2) as ph0s, \
                 tc.tile_pool(name="ph0em", bufs=1) as ph0em, \
                 tc.tile_pool(name="ph0eps", bufs=2, space="PSUM") as ph0eps, \
                 tc.tile_pool(name="ph0ps", bufs=2, space="PSUM") as ph0ps:
                nedup_sb = ph0c.tile([128, 128], bf16, tag="nedup")
                nc.sync.dma_start(nedup_sb[:], nedup_in[:])
                wftd_sb = ph0c.tile([DF + 1, 128], bf16, tag="wftd")
                nc.sync.dma_start(wftd_sb[:], wftd_in[:])
                ty_sb = ph0c.tile([1, SH], bf16, tag="ty")
                nc.sync.dma_start(ty_sb[:], typesb_in[:])
                hl0 = ph0em.tile([128, NBLK, 2 * D], bf16, tag="hl")
                # phase-0 matmuls are bf16, so emb0 is only ~4e-3 accurate;
                # the lo half adds nothing — store z0 = [hi | 0].
                nc.vector.memset(z0[64:128, :], 0.0)
                for g0 in range(0, NBLK, 4):
                    nb_ = min(4, NBLK - g0)
                    wdt = nb_ * 128
                    cols = slice(g0 * 128, g0 * 128 + wdt)
                    ft = ph0.tile([DF + 1, 512], bf16, tag="ft")
                    nc.sync.dma_start(ft[:, :wdt], featT_in[:, cols])
                    rep = ph0ps.tile([128, 512], f32, tag="rep")
                    nc.tensor.matmul(rep[:, :wdt], onesrowb_sb[:],
                                     ty_sb[:, cols], start=True, stop=True)
                    ohT = ph0.tile([128, 512], bf16, tag="ohT")
                    nc.vector.tensor_scalar(
                        ohT[:, :wdt], rep[:, :wdt], iotaP_sb[:, :1], None,
                        Alu.is_equal)
                    ps0 = ph0ps.tile([128, 512], f32, tag="ps0")
                    nc.tensor.matmul(ps0[:, :wdt], nedup_sb[:], ohT[:, :wdt],
                                     start=True, stop=False)
                    nc.tensor.matmul(ps0[:, :wdt], wftd_sb[:], ft[:, :wdt],
                                     start=False, stop=True)
                    nc.scalar.activation(z0[0:64, cols], ps0[0:64, :wdt],
                                         Act.Copy)
                    emit_blocks(z0, hl0, ph0eps, g0, nb_, on_dve=True)
                nc.sync.dma_start(shard_hl[:, :], hl0[:])

            if PHASE == "emb0":
                with tc.tile_pool(name="dbgp", bufs=1) as dp:
                    df = dp.tile([128, SH], f32, tag="df")
                    nc.vector.tensor_copy(out=df[:], in_=z0[:])
                    nc.sync.dma_start(dbg_out[:, :], df[:])

            allgather_trigger(tc, cc_ag[0])

            # ---------------- rounds -----------------------------------------
            nrounds = 0 if PHASE == "emb0" else (1 if PHASE in ("agg", "emb1") else 2)
            for r in range(nrounds):
                last = r == nrounds - 1 and PHASE == "full"
                z_prev = z0 if r == 0 else z1
                with tc.tile_pool(name=f"g{r}", bufs=GBUFS) as gpool, \
                     tc.tile_pool(name=f"oh{r}", bufs=4) as ohpool, \
                     tc.tile_pool(name=f"ps{r}", bufs=3, space="PSUM") as pspool, \
                     tc.tile_pool(name=f"x{r}", bufs=3) as xpool, \
                     tc.tile_pool(name=f"sc{r}", bufs=2) as scp, \
                     tc.tile_pool(name=f"em{r}", bufs=1) as emp, \
                     tc.tile_pool(name=f"emps{r}", bufs=2, space="PSUM") as eps, \
                     tc.tile_pool(name=f"cv{r}", bufs=2, space="PSUM") as cvps:
                    hl1 = None
                    if not last and PHASE == "full":
                        hl1 = emp.tile([128, NBLK, 2 * D], bf16, tag="hl")
                    call_tiles = {}
                    for (k, o, npos) in calls:
                        gt = gpool.tile([128, CALL // 128, 2 * D], bf16, tag="gbuf")
                        nc.gpsimd.dma_gather(
                            gt[:, :npos // 128, :],
                            table_hl[k * CH:(k + 1) * CH, :],
                            gidx_sb[:, o // 16:(o + npos) // 16],
                            npos, npos, 2 * D,
                            single_packet=SINGLE_PKT, queue_num=k)
                        call_tiles[o] = (gt, npos)
                    call_offs = sorted(call_tiles.keys())

                    import bisect

                    def tile_at(pos):
                        j = bisect.bisect_right(call_offs, pos) - 1
                        o = call_offs[j]
                        gt, npos = call_tiles[o]
                        assert o <= pos < o + npos
                        return gt[:, (pos - o) // 128, :]

                    oh_tiles = {}

                    def oh_at(t):
                        g0 = (t // G_OH) * G_OH
                        if g0 not in oh_tiles:
                            gsz = min(G_OH, NT - g0)
                            oh = ohpool.tile([128, G_OH, 128], bf16, tag="oh")
                            nc.vector.tensor_tensor(
                                out=oh[:, :gsz, :],
                                in0=iota_sb[:].rearrange(
                                    "p (g j) -> p g j", j=128)[:, :gsz, :],
                                in1=dloc_sb[:, g0:g0 + gsz].to_broadcast(
                                    [128, gsz, 128]),
                                op=Alu.is_equal)
                            oh_tiles[g0] = oh
                        return oh_tiles[g0][:, t - g0, :]

                    for g0 in range(0, NBLK, 4):
                        nb_ = min(4, NBLK - g0)
                        wdt = nb_ * 128
                        cols = slice(g0 * 128, g0 * 128 + wdt)
                        xg = xpool.tile([128, 512], f32, tag="xg")
                        for j in range(nb_):
                            b = g0 + j
                            stot = int(S_tot[b])
                            ps = pspool.tile([128, 128], f32, tag="acc")
                            for s in range(stot):
                                t = int(blk_t0[b]) + s
                                nc.tensor.matmul(
                                    ps[:], tile_at(int(cons_pos[t])), oh_at(t),
                                    start=(s == 0), stop=False)
                            # += emb (z_prev block) via identity matmul
                            nc.tensor.matmul(
                                ps[:], identb_sb[:],
                                z_prev[:, b * 128:(b + 1) * 128],
                                start=False, stop=True)
                            nc.scalar.activation(
                                xg[:, j * 128:(j + 1) * 128], ps[:], Act.Copy)

                        if PHASE == "agg" and r == 0:
                            nc.sync.dma_start(dbg_out[:, cols], xg[:, :wdt])

                        zps = cvps.tile([128, 512], f32, tag="z")
                        nc.tensor.matmul(zps[:, :wdt], wct4_sb[:], xg[:, :wdt],
                                         start=True, stop=True)
                        if not last:
                            split_hilo(z1, zps, scp, cols, wdt, relu=True)
                            if PHASE == "full":
                                emit_blocks(z1, hl1, eps, g0, nb_)
                        else:
                            zz = scp.tile([128, 512], f32, tag="zz")
                            nc.scalar.activation(
                                zz[:, :wdt], zps[:, :wdt], Act.Relu,
                                bias=cb2_sb[:, :1])
                            nc.vector.tensor_tensor(
                                out=zz[64:128, :wdt], in0=zz[64:128, :wdt],
                                in1=zz[64:128, :wdt], op=Alu.mult)
                            nq = cvps.tile([128, 8], f32, tag="nq")
                            for j in range(nb_):
                                nc.tensor.matmul(
                                    nq[:, 2 * j:2 * j + 2],
                                    zz[:, j * 128:(j + 1) * 128], pq_sb[:],
                                    start=True, stop=True)
                            nqr = nq[:].rearrange("p (b two) -> p b two", two=2)
                            nc.vector.tensor_copy(
                                out=num_slab[:, g0:g0 + nb_],
                                in_=nqr[:, :nb_, 0])
                            nc.vector.tensor_copy(
                                out=nsq_slab[:, g0:g0 + nb_],
                                in_=nqr[:, :nb_, 1])

                    if not last and PHASE == "full":
                        nc.sync.dma_start(shard_hl[:, :], hl1[:])

                if PHASE == "emb1" and r == 0:
                    with tc.tile_pool(name="dbgp", bufs=1) as dp:
                        df = dp.tile([128, SH], f32, tag="df")
                        nc.vector.tensor_copy(out=df[:], in_=z1[:])
                        nc.sync.dma_start(dbg_out[:, :], df[:])

                if not last and PHASE == "full" and r == 0 and nrounds == 2:
                    allgather_trigger(tc, cc_ag[1])

            # ---------------- cosine scores ----------------------------------
            if PHASE == "full":
                with tc.tile_pool(name="cos", bufs=1) as cos, \
                     tc.tile_pool(name="cosps", bufs=2, space="PSUM") as cosps:
                    # pnorm = max(||p||, eps) replicated to [128,1]
                    psq = cos.tile([128, 1], f32, tag="psq")
                    nc.vector.tensor_tensor(
                        out=psq[:], in0=pq_sb[:, 0:1], in1=pq_sb[:, 0:1],
                        op=Alu.mult)
                    pn_ps = cosps.tile([1, 1], f32, tag="pn")
                    nc.tensor.matmul(pn_ps[:], psq[:], ones128_sb[:],
                                     start=True, stop=True)
                    pn_sb = cos.tile([1, 1], f32, tag="pnsb")
                    nc.scalar.activation(pn_sb[:], pn_ps[:], Act.Sqrt)
                    nc.vector.tensor_scalar(
                        pn_sb[:], pn_sb[:], EPS, None, Alu.max)
                    pnr_ps = cosps.tile([128, 1], f32, tag="pnr")
                    nc.tensor.matmul(pnr_ps[:], onesrow_sb[:], pn_sb[:],
                                     start=True, stop=True)
                    pnrep = cos.tile([128, 1], f32, tag="pnrep")
                    nc.vector.tensor_copy(out=pnrep[:], in_=pnr_ps[:])

                    norm = cos.tile([128, NBLK], f32, tag="norm")
                    nc.scalar.activation(norm[:], nsq_slab[:], Act.Sqrt)
                    nc.vector.tensor_scalar(
                        norm[:], norm[:], EPS, None, Alu.max)
                    nc.vector.tensor_scalar(
                        norm[:], norm[:], pnrep[:, :1], None, Alu.mult)
                    nc.vector.reciprocal(norm[:], norm[:])
                    nc.vector.tensor_tensor(
                        out=num_slab[:], in0=num_slab[:], in1=norm[:],
                        op=Alu.mult)
                    nc.sync.dma_start(score_out[:, :], num_slab[:])
            else:
                nc.vector.memset(num_slab[:, :1], 0.0)
                nc.sync.dma_start(score_out[:, :1], num_slab[:, :1])

    nc.compile()
    return nc


# ----------------------------------------------------------------------------
# Public entry
# ----------------------------------------------------------------------------

_cache = {}


def kernel(nodes, edges, features, node_emb, feat_W, feat_b,
           conv1_W, conv1_b, pattern_emb, pattern_id):
    import ml_dtypes

    nodes = np.asarray(nodes)
    edges = np.asarray(edges)
    features = np.asarray(features, np.float32)
    node_emb = np.asarray(node_emb, np.float32)
    feat_W = np.asarray(feat_W, np.float32)
    feat_b = np.asarray(feat_b, np.float32)
    conv1_W = np.asarray(conv1_W, np.float32)
    conv1_b = np.asarray(conv1_b, np.float32)
    pattern_emb = np.asarray(pattern_emb, np.float32)
    pid = int(np.asarray(pattern_id))

    meta = _prep(nodes, edges)
    pi = meta["pi"]

    key = (meta["TPOS"], meta["S_sub"].tobytes(), PHASE)
    if key not in _cache:
        _cache.clear()
        _cache[key] = _build(meta)
    nc = _cache[key]

    types_p = np.zeros(NP, np.int64)
    types_p[pi[:N]] = nodes.astype(np.int64)
    feat_p = np.zeros((NP, DF), np.float32)
    feat_p[pi[:N]] = features

    ne_dup = np.zeros((128, 128), np.float32)
    ne_dup[:VOCAB, :D] = node_emb
    ne_dup[:VOCAB, D:] = node_emb
    wft = np.concatenate([feat_W.T, feat_b[None, :]], 0).astype(np.float32)
    wft_dup = np.tile(wft, (1, 2))
    wct4 = np.tile(conv1_W.T, (2, 2)).astype(np.float32)
    cb2 = np.tile(conv1_b, 2).reshape(128, 1).astype(np.float32)
    pq = np.zeros((128, 2), np.float32)
    pq[:D, 0] = pattern_emb[pid]
    pq[D:, 1] = 1.0
    ones128 = np.ones((128, 1), np.float32)
    ones_row = np.ones((1, 128), np.float32)
    ones_row_bf = np.ones((1, 128), dtype=ml_dtypes.bfloat16)
    iotaP = np.arange(128, dtype=np.float32).reshape(128, 1)
    identb = np.eye(128, dtype=ml_dtypes.bfloat16)
    iota_rep = np.broadcast_to(np.arange(128).astype(ml_dtypes.bfloat16),
                               (128, G_OH, 128)).reshape(128, G_OH * 128).copy()

    in_maps = []
    for c in range(W):
        rows = slice(c * SH, (c + 1) * SH)
        # column j = b*128 + p (block-major); table row = p*NBLK + b
        tv = types_p[rows].reshape(128, NBLK).T.ravel().astype(np.float32)
        fv = feat_p[rows].reshape(128, NBLK, DF).transpose(1, 0, 2)
        featT_c = fv.reshape(SH, DF).T
        featT_c = np.concatenate([featT_c, np.ones((1, SH), np.float32)], 0)
        in_maps.append({
            "featT": np.ascontiguousarray(featT_c).astype(ml_dtypes.bfloat16),
            "types_bf": tv.reshape(1, SH).astype(ml_dtypes.bfloat16),
            "ne_dup": ne_dup.astype(ml_dtypes.bfloat16),
            "wft_dup": wft_dup.astype(ml_dtypes.bfloat16), "wct4": wct4,
            "cb2": cb2, "pq": pq, "ones128": ones128,
            "ones_row": ones_row, "ones_row_bf": ones_row_bf,
            "iotaP": iotaP, "identb": identb,
            "iota_rep": iota_rep,
            "gidx16": meta["gidx16"][c],
            "dloc_c": meta["dloc_c"][c],
        })

    tdir = os.environ.get("BASS_GNN_TRACE_DIR") or None
    res = run_bass_kernel_spmd(nc, in_maps, core_ids=list(range(W)),
                               trace=TRACE, tmpdir=tdir)
    kernel.last_results = res

    if PHASE != "full":
        dump = np.stack([res.results[c]["dbg"] for c in range(W)], 0)
        return dump

    out_p = np.empty(NP, np.float32)
    for c in range(W):
        s = res.results[c]["score"]
        out_p[c * SH:(c + 1) * SH] = s.reshape(SH)
    return out_p[pi[:N]]


# revision 36
# speedup vs baseline: 1.0213x; 1.0213x over previous
"""GNN message-passing kernel for Trainium2 (8 NeuronCores, Bass/Tile).

Implements:
  embeds = node_emb[nodes] + features @ feat_W.T + feat_b
  2 x { agg[dst] += embeds[src];  embeds = relu((embeds+agg) @ conv1_W.T + conv1_b) }
  out = cosine(embeds, pattern_emb[pattern_id])

Distribution: nodes are permuted (in-degree balanced) and sharded over 8 cores
(12544 nodes each).  Edges are partitioned by dst owner; messages are fetched
with GPSIMD dma_gather (4 SWDGE queues) from a replicated bf16 hi|lo DRAM
table rebuilt each round with an ncfw AllGather.

Data layout: everything on-chip is FEATURE-MAJOR with the hi|lo split on
partitions (0:64 = hi contribution, 64:128 = lo contribution); columns are
nodes in (block, slot) order, col = b*128 + p.  The segment-sum matmul is
  psum[hl, dst] += tile[edge, hl]^T @ onehot[edge, dst]
and the hi+lo fold happens for free inside the conv matmul by duplicating
conv1_W rows (wct4).  The only transposes are 98 PE transposes per table
emit (feature-major -> node-major rows).
"""

import os
import sys

sys.path.insert(0, "/opt/trn_rl_repo")

import numpy as np

import concourse.bass as bass
import concourse.bacc as bacc
import concourse.mybir as mybir
import concourse.tile as tile
from concourse import library_config
from concourse.bass_utils import run_bass_kernel_spmd

N = 100000
NP = 100352          # padded: 8 * 12544
W = 8                # cores
SH = NP // W         # 12544 nodes per core
NBLK = SH // 128     # 98 blocks per core
NBG = NP // 128      # 784 global blocks
NCHUNK = 4
CH = NP // NCHUNK    # 25088 rows per gather chunk (int16-safe)
D = 64
DF = 10
VOCAB = 100
EPS = 1e-8
CALL = int(os.environ.get("BASS_GNN_CALL", "768"))   # gather positions per dma_gather call
G_OH = 16            # one-hot tiles built per DVE op
PHASE = os.environ.get("BASS_GNN_PHASE", "full")
TRACE = os.environ.get("BASS_GNN_TRACE", "0") == "1"
GBUFS = int(os.environ.get("BASS_GNN_GBUFS", "24"))
SINGLE_PKT = os.environ.get("BASS_GNN_SINGLE_PKT", "1") == "1"

f32 = mybir.dt.float32
bf16 = mybir.dt.bfloat16
i16 = mybir.dt.int16
Alu = mybir.AluOpType
Act = mybir.ActivationFunctionType


# ----------------------------------------------------------------------------
# Host-side preprocessing: permutation, block assignment, edge streams.
# ----------------------------------------------------------------------------

def _prep(nodes, edges):
    src = edges[:, 0].astype(np.int64)
    dst = edges[:, 1].astype(np.int64)

    indeg = np.zeros(NP, np.int64)
    np.add.at(indeg, dst, 1)
    order = np.argsort(-indeg, kind="stable")
    # round-robin by in-degree over 784 global blocks -> balanced block loads
    gblock = np.empty(NP, np.int64)
    gslot = np.empty(NP, np.int64)
    gblock[order] = np.arange(NP) % NBG
    gslot[order] = np.arange(NP) // NBG

    # pass 1 fixes each node's core (g % W), hence its gather chunk as a src
    # (core pair = c//2).  Pass 2 re-bins nodes WITHIN each core into blocks,
    # greedily capping every (block, chunk) in-edge cell at <=2 tiles (256),
    # which both shrinks S_sub and aligns tile needs across cores.
    core_of_gblock = np.arange(NBG) % W
    c_n = core_of_gblock[gblock]
    prof = np.zeros((NP, NCHUNK), np.int64)
    np.add.at(prof, (dst, c_n[src] // 2), 1)

    b_n = np.empty(NP, np.int64)
    p_n = np.empty(NP, np.int64)
    for c in range(W):
        mine = np.where(c_n == c)[0]
        mine = mine[np.argsort(-prof[mine].sum(1), kind="stable")]
        loads = np.zeros((NBLK, NCHUNK), np.int64)
        fill = np.zeros(NBLK, np.int64)
        for n in mine:
            cand = loads + prof[n]
            # penalize only NEW tile-count increases past 2 -> overflow
            # concentrates in a few already-heavy cells instead of spreading
            cur = np.maximum(-(-loads // 128), 2)
            tinc = np.maximum(-(-cand // 128) - cur, 0).sum(1)
            score = tinc * 1e6 + cand.max(1) * 1.0 + fill * 1e-3
            score[fill >= 128] = np.inf
            b = int(np.argmin(score))
            b_n[n] = b
            p_n[n] = fill[b]
            fill[b] += 1
            loads[b] += prof[n]
        # align the residual >2-tile cells across cores: relabel blocks by
        # their tile-need pattern (same key as the old alignment sort)
        t0 = np.ceil(loads / 128).astype(np.int64)
        key = (t0 * (5 ** np.arange(NCHUNK))[::-1]).sum(1) * 1000 + loads.sum(1) // 16
        relabel = np.empty(NBLK, np.int64)
        relabel[np.argsort(key, kind="stable")] = np.arange(NBLK)
        b_n[mine] = relabel[b_n[mine]]

    # final permutation: pi(n) = core*SH + slot_within_block*NBLK + local_block
    # (p-major layout so SBUF slab [128, 98, 128] maps contiguously to DRAM rows)
    pi = c_n * SH + p_n * NBLK + b_n

    # exact per-(core, local block, chunk) edge counts under final assignment
    e_dc = c_n[dst]
    e_db = b_n[dst]
    e_dp = p_n[dst]
    e_chunk = pi[src] // CH
    Ec = np.zeros((W, NBLK, NCHUNK), np.int64)
    np.add.at(Ec, (e_dc, e_db, e_chunk), 1)
    S_sub = np.ceil(Ec / 128).astype(np.int64).max(0)  # [NBLK, NCHUNK]
    S_sub = np.maximum(S_sub, 1)   # keep schedule uniform & non-degenerate
    assert S_sub.max() <= CALL // 128

    # edge order: (core, block, chunk, src)
    ek = np.lexsort((pi[src], e_chunk, e_db, e_dc))
    src_pi = pi[src][ek]
    dloc_e = e_dp[ek]
    key_cbk = (e_dc[ek] * NBLK + e_db[ek]) * NCHUNK + e_chunk[ek]
    counts = np.zeros(W * NBLK * NCHUNK + 1, np.int64)
    np.add.at(counts, key_cbk + 1, 1)
    starts = np.cumsum(counts)

    # gather-stream offsets: chunk-major, then block, then tile
    TPOS = int(S_sub.sum() * 128)
    NT = TPOS // 128
    sched_off = np.zeros((NCHUNK, NBLK), np.int64)
    off = 0
    for k in range(NCHUNK):
        for b in range(NBLK):
            sched_off[k, b] = off
            off += int(S_sub[b, k]) * 128
    assert off == TPOS

    gidx = np.zeros((W, TPOS), np.int32)
    dloc = np.full((W, TPOS), -1.0, np.float32)   # stream order
    for c in range(W):
        for k in range(NCHUNK):
            for b in range(NBLK):
                i0 = starts[(c * NBLK + b) * NCHUNK + k]
                i1 = starts[(c * NBLK + b) * NCHUNK + k + 1]
                n = i1 - i0
                o = sched_off[k, b]
                assert n <= S_sub[b, k] * 128
                gidx[c, o:o + n] = src_pi[i0:i1] - k * CH
                dloc[c, o:o + n] = dloc_e[i0:i1]
    assert gidx.min() >= 0 and gidx.max() < CH

    # wrapped-16 int16 idx layout (stream order)
    gidx16 = np.zeros((W, 128, TPOS // 16), np.int16)
    for c in range(W):
        g = gidx[c].reshape(-1, 16).T.astype(np.int16)
        gidx16[c] = np.tile(g, (8, 1))

    # consumption order: block-major (b, k, s) -> stream position of each tile
    cons_pos = []
    for b in range(NBLK):
        for k in range(NCHUNK):
            for s in range(int(S_sub[b, k])):
                cons_pos.append(int(sched_off[k, b]) + s * 128)
    cons_pos = np.array(cons_pos, np.int64)
    assert len(cons_pos) == NT
    # dloc in consumption-tile-column layout [128, NT]
    dloc_s = dloc.reshape(W, NT, 128)        # stream tile t at rows [t]
    stream_tile = cons_pos // 128
    dloc_c = dloc_s[:, stream_tile, :].transpose(0, 2, 1).astype(np.float32).copy()

    # per-chunk call plan, issued round-robin across chunks
    ncalls_k = []
    base_k = []
    for k in range(NCHUNK):
        npos = int(S_sub[:, k].sum() * 128)
        base_k.append(int(sched_off[k, 0]))
        ncalls_k.append((npos + CALL - 1) // CALL)
    calls = []   # (k, stream_off, npos) in issue order
    for ci in range(max(ncalls_k)):
        for k in range(NCHUNK):
            if ci < ncalls_k[k]:
                npos_k = int(S_sub[:, k].sum() * 128)
                o = base_k[k] + ci * CALL
                n = min(CALL, base_k[k] + npos_k - o)
                calls.append((k, o, n))

    return {
        "pi": pi, "S_sub": S_sub, "TPOS": TPOS, "NT": NT,
        "gidx16": gidx16, "dloc_c": dloc_c, "calls": calls,
        "sched_off": sched_off, "cons_pos": cons_pos,
    }


# ----------------------------------------------------------------------------
# Kernel builder
# ----------------------------------------------------------------------------

def _build(meta):
    S_sub = meta["S_sub"]
    TPOS = meta["TPOS"]
    NT = meta["NT"]
    calls = meta["calls"]
    cons_pos = meta["cons_pos"]

    nc = bacc.Bacc("TRN2", debug=False, num_swdge_queues=4)

    featT_in = nc.dram_tensor("featT", [DF + 1, SH], bf16, kind="ExternalInput")
    nedup_in = nc.dram_tensor("ne_dup", [128, 128], bf16, kind="ExternalInput")
    wftd_in = nc.dram_tensor("wft_dup", [DF + 1, 128], bf16, kind="ExternalInput")
    wct4_in = nc.dram_tensor("wct4", [128, 128], f32, kind="ExternalInput")
    cb2_in = nc.dram_tensor("cb2", [128, 1], f32, kind="ExternalInput")
    pq_in = nc.dram_tensor("pq", [128, 2], f32, kind="ExternalInput")
    ones128_in = nc.dram_tensor("ones128", [128, 1], f32, kind="ExternalInput")
    onesrow_in = nc.dram_tensor("ones_row", [1, 128], f32, kind="ExternalInput")
    onesrowb_in = nc.dram_tensor("ones_row_bf", [1, 128], bf16, kind="ExternalInput")
    typesb_in = nc.dram_tensor("types_bf", [1, SH], bf16, kind="ExternalInput")
    iotaP_in = nc.dram_tensor("iotaP", [128, 1], f32, kind="ExternalInput")
    identb_in = nc.dram_tensor("identb", [128, 128], bf16, kind="ExternalInput")
    iota_in = nc.dram_tensor("iota_rep", [128, G_OH * 128], bf16, kind="ExternalInput")
    gidx_in = nc.dram_tensor("gidx16", [128, TPOS // 16], i16, kind="ExternalInput")
    dloc_in = nc.dram_tensor("dloc_c", [128, NT], f32, kind="ExternalInput")

    score_out = nc.dram_tensor("score", [128, NBLK], f32, kind="ExternalOutput")
    if PHASE in ("emb0", "agg", "emb1"):
        dbg_out = nc.dram_tensor("dbg", [128, SH], f32, kind="ExternalOutput")

    shard_hl = nc.dram_tensor("shard_hl", [SH, 2 * D], bf16)
    table_hl = nc.dram_tensor("table_hl", [NP, 2 * D], bf16, addr_space="Shared")

    import contextlib

    def allgather_trigger(tcx, cc):
        """Run the AllGather inside a critical section (deferring the wait
        to the consumer stream deadlocks the tile scheduler)."""
        with tcx.tile_critical(no_gpsimd_drain=True):
            nc.gpsimd.collective_compute(
                "AllGather", Alu.bypass,
                replica_groups=[list(range(W))],
                ins=[shard_hl.ap().opt()],
                outs=[table_hl.ap().opt()],
            ).then_inc(cc, 1)
            nc.gpsimd.wait_ge(cc, 1)

    # consumption-tile schedule per block: (first_tile_index, total_tiles)
    S_tot = S_sub.sum(1).astype(np.int64)
    blk_t0 = np.concatenate([[0], np.cumsum(S_tot)])[:-1]

    with tile.TileContext(nc) as tc:
        nc.gpsimd.load_library(library_config.mlp)
        with contextlib.ExitStack() as big:
            cpool = big.enter_context(tc.tile_pool(name="consts", bufs=1))
            cc_ag0 = big.enter_context(nc.semaphore())
            cc_ag1 = big.enter_context(nc.semaphore())
            cc_ag = [cc_ag0, cc_ag1]
            slabs = big.enter_context(tc.tile_pool(name="slabs", bufs=1))

            wct4_sb = cpool.tile([128, 128], f32, tag="wct4")
            nc.sync.dma_start(wct4_sb[:], wct4_in[:])
            cb2_sb = cpool.tile([128, 1], f32, tag="cb2")
            nc.sync.dma_start(cb2_sb[:], cb2_in[:])
            pq_sb = cpool.tile([128, 2], f32, tag="pq")
            nc.sync.dma_start(pq_sb[:], pq_in[:])
            ones128_sb = cpool.tile([128, 1], f32, tag="ones128")
            nc.sync.dma_start(ones128_sb[:], ones128_in[:])
            onesrow_sb = cpool.tile([1, 128], f32, tag="onesrow")
            nc.sync.dma_start(onesrow_sb[:], onesrow_in[:])
            onesrowb_sb = cpool.tile([1, 128], bf16, tag="onesrowb")
            nc.sync.dma_start(onesrowb_sb[:], onesrowb_in[:])
            iotaP_sb = cpool.tile([128, 1], f32, tag="iotaP")
            nc.sync.dma_start(iotaP_sb[:], iotaP_in[:])
            identb_sb = cpool.tile([128, 128], bf16, tag="identb")
            nc.sync.dma_start(identb_sb[:], identb_in[:])
            iota_sb = cpool.tile([128, G_OH * 128], bf16, tag="iota")
            nc.sync.dma_start(iota_sb[:], iota_in[:])
            dloc_sb = cpool.tile([128, NT], f32, tag="dloc")
            nc.sync.dma_start(dloc_sb[:], dloc_in[:])
            gidx_sb = cpool.tile([128, TPOS // 16], i16, tag="gidx")
            nc.sync.dma_start(gidx_sb[:], gidx_in[:])

            z0 = slabs.tile([128, SH], bf16, tag="z0")
            z1 = slabs.tile([128, SH], bf16, tag="z1")
            num_slab = slabs.tile([128, NBLK], f32, tag="num")
            nsq_slab = slabs.tile([128, NBLK], f32, tag="nsq")

            def split_hilo(z_slab, src_ps, scp_, cols, wdt, relu):
                """Exact hi|lo split of src_ps into z_slab[:, cols] (bf16).

                hi half (partitions 0:64) and lo half (64:128) of src_ps hold
                the same value v (replicated); store hi = bf16(v) and
                lo = bf16(v - f32(hi)).  Copies/casts run on the Scalar
                engine; subtract on DVE.
                """
                if relu:
                    sf = scp_.tile([128, 512], f32, tag="sf")
                    nc.scalar.activation(sf[:, :wdt], src_ps[:, :wdt],
                                         Act.Relu, bias=cb2_sb[:, :1])
                    v = sf
                else:
                    v = src_ps
                nc.scalar.activation(z_slab[0:64, cols], v[0:64, :wdt],
                                     Act.Copy)
                sb_ = scp_.tile([128, 512], bf16, tag="sb")
                nc.vector.tensor_copy(out=sb_[64:128, :wdt],
                                      in_=v[64:128, :wdt])
                sf2 = scp_.tile([128, 512], f32, tag="sf2")
                nc.scalar.activation(sf2[64:128, :wdt], sb_[64:128, :wdt],
                                     Act.Copy)
                nc.vector.tensor_tensor(
                    out=sf2[64:128, :wdt], in0=v[64:128, :wdt],
                    in1=sf2[64:128, :wdt], op=Alu.subtract)
                nc.scalar.activation(z_slab[64:128, cols], sf2[64:128, :wdt],
                                     Act.Copy)

            def emit_blocks(z_slab, hl, eps, b0, nblks, on_dve=False):
                """PE-transpose blocks b0..b0+nblks of z into the hl slab."""
                for b in range(b0, b0 + nblks):
                    tp = eps.tile([128, 128], bf16, tag="tp")
                    nc.tensor.transpose(
                        tp[:], z_slab[:, b * 128:(b + 1) * 128], identb_sb[:])
                    if on_dve:
                        nc.vector.tensor_copy(out=hl[:, b, :], in_=tp[:])
                    else:
                        nc.scalar.activation(hl[:, b, :], tp[:], Act.Copy)

            # ---------------- Phase 0: initial embeddings (feature-major) ---
            with tc.tile_pool(name="ph0", bufs=3) as ph0, \
                 tc.tile_pool(name="ph0c", bufs=1) as ph0c, \
                 tc.tile_pool(name="ph0s", bufs=2) as ph0s, \
                 tc.tile_pool(name="ph0em", bufs=1) as ph0em, \
                 tc.tile_pool(name="ph0eps", bufs=2, space="PSUM") as ph0eps, \
                 tc.tile_pool(name="ph0ps", bufs=2, space="PSUM") as ph0ps:
                nedup_sb = ph0c.tile([128, 128], bf16, tag="nedup")
                nc.sync.dma_start(nedup_sb[:], nedup_in[:])
                wftd_sb = ph0c.tile([DF + 1, 128], bf16, tag="wftd")
                nc.sync.dma_start(wftd_sb[:], wftd_in[:])
                ty_sb = ph0c.tile([1, SH], bf16, tag="ty")
                nc.sync.dma_start(ty_sb[:], typesb_in[:])
                hl0 = ph0em.tile([128, NBLK, 2 * D], bf16, tag="hl")
                nc.vector.memset(hl0[:, :, D:], 0.0)
                # phase-0 matmuls are bf16, so emb0 is only ~4e-3 accurate;
                # the lo half adds nothing — store z0 = [hi | 0].
                nc.vector.memset(z0[64:128, :], 0.0)
                for g0 in range(0, NBLK, 4):
                    nb_ = min(4, NBLK - g0)
                    wdt = nb_ * 128
                    cols = slice(g0 * 128, g0 * 128 + wdt)
                    ft = ph0.tile([DF + 1, 512], bf16, tag="ft")
                    nc.sync.dma_start(ft[:, :wdt], featT_in[:, cols])
                    rep = ph0ps.tile([128, 512], f32, tag="rep")
                    nc.tensor.matmul(rep[:, :wdt], onesrowb_sb[:],
                                     ty_sb[:, cols], start=True, stop=True)
                    ohT = ph0.tile([128, 512], bf16, tag="ohT")
                    nc.vector.tensor_scalar(
                        ohT[:, :wdt], rep[:, :wdt], iotaP_sb[:, :1], None,
                        Alu.is_equal)
                    ps0 = ph0ps.tile([128, 512], f32, tag="ps0")
                    nc.tensor.matmul(ps0[:, :wdt], nedup_sb[:], ohT[:, :wdt],
                                     start=True, stop=False)
                    nc.tensor.matmul(ps0[:, :wdt], wftd_sb[:], ft[:, :wdt],
                                     start=False, stop=True)
                    nc.scalar.activation(z0[0:64, cols], ps0[0:64, :wdt],
                                         Act.Copy)
                    # z0 lo is zero: transpose only the hi half [64,128]
                    for b in range(g0, g0 + nb_):
                        tp = ph0eps.tile([128, 64], bf16, tag="tp")
                        nc.tensor.transpose(
                            tp[:], z0[0:64, b * 128:(b + 1) * 128],
                            identb_sb[0:64, 0:64])
                        nc.vector.tensor_copy(out=hl0[:, b, 0:D], in_=tp[:])
                nc.sync.dma_start(shard_hl[:, :], hl0[:])

            if PHASE == "emb0":
                with tc.tile_pool(name="dbgp", bufs=1) as dp:
                    df = dp.tile([128, SH], f32, tag="df")
                    nc.vector.tensor_copy(out=df[:], in_=z0[:])
                    nc.sync.dma_start(dbg_out[:, :], df[:])

            allgather_trigger(tc, cc_ag[0])

            # ---------------- rounds -----------------------------------------
            nrounds = 0 if PHASE == "emb0" else (1 if PHASE in ("agg", "emb1") else 2)
            for r in range(nrounds):
                last = r == nrounds - 1 and PHASE == "full"
                z_prev = z0 if r == 0 else z1
                with tc.tile_pool(name=f"g{r}", bufs=GBUFS) as gpool, \
                     tc.tile_pool(name=f"oh{r}", bufs=4) as ohpool, \
                     tc.tile_pool(name=f"ps{r}", bufs=3, space="PSUM") as pspool, \
                     tc.tile_pool(name=f"x{r}", bufs=3) as xpool, \
                     tc.tile_pool(name=f"sc{r}", bufs=2) as scp, \
                     tc.tile_pool(name=f"em{r}", bufs=1) as emp, \
                     tc.tile_pool(name=f"emps{r}", bufs=2, space="PSUM") as eps, \
                     tc.tile_pool(name=f"cv{r}", bufs=2, space="PSUM") as cvps:
                    hl1 = None
                    if not last and PHASE == "full":
                        hl1 = emp.tile([128, NBLK, 2 * D], bf16, tag="hl")
                    call_tiles = {}
                    for (k, o, npos) in calls:
                        gt = gpool.tile([128, CALL // 128, 2 * D], bf16, tag="gbuf")
                        nc.gpsimd.dma_gather(
                            gt[:, :npos // 128, :],
                            table_hl[k * CH:(k + 1) * CH, :],
                            gidx_sb[:, o // 16:(o + npos) // 16],
                            npos, npos, 2 * D,
                            single_packet=SINGLE_PKT, queue_num=k)
                        call_tiles[o] = (gt, npos)
                    call_offs = sorted(call_tiles.keys())

                    import bisect

                    def tile_at(pos):
                        j = bisect.bisect_right(call_offs, pos) - 1
                        o = call_offs[j]
                        gt, npos = call_tiles[o]
                        assert o <= pos < o + npos
                        return gt[:, (pos - o) // 128, :]

                    oh_tiles = {}

                    def oh_at(t):
                        g0 = (t // G_OH) * G_OH
                        if g0 not in oh_tiles:
                            gsz = min(G_OH, NT - g0)
                            oh = ohpool.tile([128, G_OH, 128], bf16, tag="oh")
                            nc.vector.tensor_tensor(
                                out=oh[:, :gsz, :],
                                in0=iota_sb[:].rearrange(
                                    "p (g j) -> p g j", j=128)[:, :gsz, :],
                                in1=dloc_sb[:, g0:g0 + gsz].to_broadcast(
                                    [128, gsz, 128]),
                                op=Alu.is_equal)
                            oh_tiles[g0] = oh
                        return oh_tiles[g0][:, t - g0, :]

                    for g0 in range(0, NBLK, 4):
                        nb_ = min(4, NBLK - g0)
                        wdt = nb_ * 128
                        cols = slice(g0 * 128, g0 * 128 + wdt)
                        xg = xpool.tile([128, 512], f32, tag="xg")
                        for j in range(nb_):
                            b = g0 + j
                            stot = int(S_tot[b])
                            ps = pspool.tile([128, 128], f32, tag="acc")
                            for s in range(stot):
                                t = int(blk_t0[b]) + s
                                nc.tensor.matmul(
                                    ps[:], tile_at(int(cons_pos[t])), oh_at(t),
                                    start=(s == 0), stop=False)
                            # += emb (z_prev block) via identity matmul
                            nc.tensor.matmul(
                                ps[:], identb_sb[:],
                                z_prev[:, b * 128:(b + 1) * 128],
                                start=False, stop=True)
                            nc.scalar.activation(
                                xg[:, j * 128:(j + 1) * 128], ps[:], Act.Copy)

                        if PHASE == "agg" and r == 0:
                            nc.sync.dma_start(dbg_out[:, cols], xg[:, :wdt])

                        zps = cvps.tile([128, 512], f32, tag="z")
                        nc.tensor.matmul(zps[:, :wdt], wct4_sb[:], xg[:, :wdt],
                                         start=True, stop=True)
                        if not last:
                            split_hilo(z1, zps, scp, cols, wdt, relu=True)
                            if PHASE == "full":
                                emit_blocks(z1, hl1, eps, g0, nb_)
                        else:
                            zz = scp.tile([128, 512], f32, tag="zz")
                            nc.scalar.activation(
                                zz[:, :wdt], zps[:, :wdt], Act.Relu,
                                bias=cb2_sb[:, :1])
                            nc.vector.tensor_tensor(
                                out=zz[64:128, :wdt], in0=zz[64:128, :wdt],
                                in1=zz[64:128, :wdt], op=Alu.mult)
                            nq = cvps.tile([128, 8], f32, tag="nq")
                            for j in range(nb_):
                                nc.tensor.matmul(
                                    nq[:, 2 * j:2 * j + 2],
                                    zz[:, j * 128:(j + 1) * 128], pq_sb[:],
                                    start=True, stop=True)
                            nqr = nq[:].rearrange("p (b two) -> p b two", two=2)
                            nc.vector.tensor_copy(
                                out=num_slab[:, g0:g0 + nb_],
                                in_=nqr[:, :nb_, 0])
                            nc.vector.tensor_copy(
                                out=nsq_slab[:, g0:g0 + nb_],
                                in_=nqr[:, :nb_, 1])

                    if not last and PHASE == "full":
                        nc.sync.dma_start(shard_hl[:, :], hl1[:])

                if PHASE == "emb1" and r == 0:
                    with tc.tile_pool(name="dbgp", bufs=1) as dp:
                        df = dp.tile([128, SH], f32, tag="df")
                        nc.vector.tensor_copy(out=df[:], in_=z1[:])
                        nc.sync.dma_start(dbg_out[:, :], df[:])

                if not last and PHASE == "full" and r == 0 and nrounds == 2:
                    allgather_trigger(tc, cc_ag[1])

            # ---------------- cosine scores ----------------------------------
            if PHASE == "full":
                with tc.tile_pool(name="cos", bufs=1) as cos, \
                     tc.tile_pool(name="cosps", bufs=2, space="PSUM") as cosps:
                    # pnorm = max(||p||, eps) replicated to [128,1]
                    psq = cos.tile([128, 1], f32, tag="psq")
                    nc.vector.tensor_tensor(
                        out=psq[:], in0=pq_sb[:, 0:1], in1=pq_sb[:, 0:1],
                        op=Alu.mult)
                    pn_ps = cosps.tile([1, 1], f32, tag="pn")
                    nc.tensor.matmul(pn_ps[:], psq[:], ones128_sb[:],
                                     start=True, stop=True)
                    pn_sb = cos.tile([1, 1], f32, tag="pnsb")
                    nc.scalar.activation(pn_sb[:], pn_ps[:], Act.Sqrt)
                    nc.vector.tensor_scalar(
                        pn_sb[:], pn_sb[:], EPS, None, Alu.max)
                    pnr_ps = cosps.tile([128, 1], f32, tag="pnr")
                    nc.tensor.matmul(pnr_ps[:], onesrow_sb[:], pn_sb[:],
                                     start=True, stop=True)
                    pnrep = cos.tile([128, 1], f32, tag="pnrep")
                    nc.vector.tensor_copy(out=pnrep[:], in_=pnr_ps[:])

                    norm = cos.tile([128, NBLK], f32, tag="norm")
                    nc.scalar.activation(norm[:], nsq_slab[:], Act.Sqrt)
                    nc.vector.tensor_scalar(
                        norm[:], norm[:], EPS, None, Alu.max)
                    nc.vector.tensor_scalar(
                        norm[:], norm[:], pnrep[:, :1], None, Alu.mult)
                    nc.vector.reciprocal(norm[:], norm[:])
                    nc.vector.tensor_tensor(
                        out=num_slab[:], in0=num_slab[:], in1=norm[:],
                        op=Alu.mult)
                    nc.sync.dma_start(score_out[:, :], num_slab[:])
            else:
                nc.vector.memset(num_slab[:, :1], 0.0)
                nc.sync.dma_start(score_out[:, :1], num_slab[:, :1])

    nc.compile()
    return nc


# ----------------------------------------------------------------------------
# Public entry
# ----------------------------------------------------------------------------

_cache = {}


def kernel(nodes, edges, features, node_emb, feat_W, feat_b,
           conv1_W, conv1_b, pattern_emb, pattern_id):
    import ml_dtypes

    nodes = np.asarray(nodes)
    edges = np.asarray(edges)
    features = np.asarray(features, np.float32)
    node_emb = np.asarray(node_emb, np.float32)
    feat_W = np.asarray(feat_W, np.float32)
    feat_b = np.asarray(feat_b, np.float32)
    conv1_W = np.asarray(conv1_W, np.float32)
    conv1_b = np.asarray(conv1_b, np.float32)
    pattern_emb = np.asarray(pattern_emb, np.float32)
    pid = int(np.asarray(pattern_id))

    meta = _prep(nodes, edges)
    pi = meta["pi"]

    key = (meta["TPOS"], meta["S_sub"].tobytes(), PHASE)
    if key not in _cache:
        _cache.clear()
        _cache[key] = _build(meta)
    nc = _cache[key]

    types_p = np.zeros(NP, np.int64)
    types_p[pi[:N]] = nodes.astype(np.int64)
    feat_p = np.zeros((NP, DF), np.float32)
    feat_p[pi[:N]] = features

    ne_dup = np.zeros((128, 128), np.float32)
    ne_dup[:VOCAB, :D] = node_emb
    ne_dup[:VOCAB, D:] = node_emb
    wft = np.concatenate([feat_W.T, feat_b[None, :]], 0).astype(np.float32)
    wft_dup = np.tile(wft, (1, 2))
    wct4 = np.tile(conv1_W.T, (2, 2)).astype(np.float32)
    cb2 = np.tile(conv1_b, 2).reshape(128, 1).astype(np.float32)
    pq = np.zeros((128, 2), np.float32)
    pq[:D, 0] = pattern_emb[pid]
    pq[D:, 1] = 1.0
    ones128 = np.ones((128, 1), np.float32)
    ones_row = np.ones((1, 128), np.float32)
    ones_row_bf = np.ones((1, 128), dtype=ml_dtypes.bfloat16)
    iotaP = np.arange(128, dtype=np.float32).reshape(128, 1)
    identb = np.eye(128, dtype=ml_dtypes.bfloat16)
    iota_rep = np.broadcast_to(np.arange(128).astype(ml_dtypes.bfloat16),
                               (128, G_OH, 128)).reshape(128, G_OH * 128).copy()

    in_maps = []
    for c in range(W):
        rows = slice(c * SH, (c + 1) * SH)
        # column j = b*128 + p (block-major); table row = p*NBLK + b
        tv = types_p[rows].reshape(128, NBLK).T.ravel().astype(np.float32)
        fv = feat_p[rows].reshape(128, NBLK, DF).transpose(1, 0, 2)
        featT_c = fv.reshape(SH, DF).T
        featT_c = np.concatenate([featT_c, np.ones((1, SH), np.float32)], 0)
        in_maps.append({
            "featT": np.ascontiguousarray(featT_c).astype(ml_dtypes.bfloat16),
            "types_bf": tv.reshape(1, SH).astype(ml_dtypes.bfloat16),
            "ne_dup": ne_dup.astype(ml_dtypes.bfloat16),
            "wft_dup": wft_dup.astype(ml_dtypes.bfloat16), "wct4": wct4,
            "cb2": cb2, "pq": pq, "ones128": ones128,
            "ones_row": ones_row, "ones_row_bf": ones_row_bf,
            "iotaP": iotaP, "identb": identb,
            "iota_rep": iota_rep,
            "gidx16": meta["gidx16"][c],
            "dloc_c": meta["dloc_c"][c],
        })

    tdir = os.environ.get("BASS_GNN_TRACE_DIR") or None
    res = run_bass_kernel_spmd(nc, in_maps, core_ids=list(range(W)),
                               trace=TRACE, tmpdir=tdir)
    kernel.last_results = res

    if PHASE != "full":
        dump = np.stack([res.results[c]["dbg"] for c in range(W)], 0)
        return dump

    out_p = np.empty(NP, np.float32)
    for c in range(W):
        s = res.results[c]["score"]
        out_p[c * SH:(c + 1) * SH] = s.reshape(SH)
    return out_p[pi[:N]]
